# revision 26
# baseline (speedup 1.0000x reference)
"""Trainium2 Bass kernel for nn_BasicBlock (distance-transform conv BasicBlock).

Computes: relu(bn2(dt_conv2(relu(bn1(dt_conv1(x))))) + x)
where dt_conv is a 3x3 "distance transform conv":
    d[b,o,h,w] = sqrt(||p - c_o||^2),  p = 3x3 zero-padded patch (dim 576)

Strategy (8 NeuronCores, data-parallel over batch 32 -> 4 images/core):

6 matmuls per pixel-group instead of 9 (1.5x less PE work). With
||p||^2 - 2 p.c + ||c||^2, the -2p.c part needs all 9 taps x 64 ch = 576
contraction lanes, but the ||p||^2 part is predigested: v = row3(x^2)
(3-tap horizontal sum of the per-channel square, on DVE) compresses the
576 norm lanes into 192. Two SBUF planes per image:
  A = [x | x01]  (f32r, x01 = x shifted left 1 col; both direct HBM DMAs)
  B = [x02 | v]  (fp16)
Six matmuls per group, all at window column 0, rows a=0..2:
  A windows: top-> tap(a,0) w/ centers, bottom-> tap(a,1) w/ centers
  B windows: top-> tap(a,2) w/ centers, bottom-> v w/ ones (sums to ||p||^2)
K=128 fully packed: 576 + 192 = 768 = 6 x 128.

Images b and b+2 share one PSUM bank (PE tile_position col 0 / col 64), so
evictions (ACT sqrt+bias) run at full 128-lane width, N=448 (8 rows x 56).
BN batch stats via bn_stats/bn_aggr on the evicted d chunks (DVE), fold,
[64,2] AllGather across 8 cores + local rank-sum (sync-BN), x2 layers.
Layer 2 reuses A/B buffers: y (ACT glue, f32), y01 (DVE copy), y02
(GPSIMD convert-copy), v' = row3(y^2) (ACT square + DVE adds). Zero
borders are established once (rep 0) and preserved by interior-only
writes. Final: out = relu(s2*d + t2 + x): GPSIMD stt + ACT relu + DMA.

kernel(**inputs) takes FULL unsharded inputs, returns FULL output.
Self-contained: shapes/sharding hardcoded; no file reads.
"""
import numpy as np

from concourse import bacc, mybir, tile
from concourse.bass_utils import run_bass_kernel_spmd

f32 = mybir.dt.float32
f32r = mybir.dt.float32r
f16 = mybir.dt.float16
ADD = mybir.AluOpType.add
MULT = mybir.AluOpType.mult
SUB = mybir.AluOpType.subtract
AF = mybir.ActivationFunctionType

N_CORES = 8
B_LOCAL = 4            # images per core (32 / 8)
C = 64                 # channels (in == out)
HW = 56                # spatial
HP = HW + 2            # padded plane side
RPG = 8                # rows per matmul group (N = 8*56 = 448)
NG = 7                 # groups per image-pair (7*8 = 56 rows)
N_GLOBAL = 32 * HW * HW             # BN normalization count (global batch)
N_LOCAL_HALF = 2 * HW * HW          # values per partition in bn_aggr (2 imgs)
BN_EPS = 1e-5


def _build_layer(nc, psum, A, Bp, wA, wB, cst, ci, d, bnt):
    """One dt_conv layer: 14 full-bank groups x 2x6 window matmuls (N=448).
    A[b] f32r [128, 58, 58] = [x | x01]; Bp[b] f16 [128, 58, 58] = [x02 | v].
    Images b=i and b=2+i share a PSUM bank (partitions 0:64 / 64:128).
    d is [128, 2, HW, HW]; bnt [128, 14, 6] per-chunk bn stats.
    Returns eviction instructions (scheduling anchors)."""
    evicts = []
    # groups fused in pairs per PSUM tile (2 banks) so each ACT eviction
    # covers 16 rows; 7 groups -> (0,1) (2,3) (4,5) (6)
    fused = [(0, 1), (2, 3), (4, 5), (6,)]
    for i in range(2):              # image-pair index (b = 2*h + i)
        for gs in fused:
            for h in range(2):
                b = 2 * h + i
                pb = 64 * h
                # matmul output must start at PSUM partition 0 (ISA rule),
                # so each image gets 64-wide banks; the eviction shifts
                # partitions (ACT out base pb) into the 128-wide d.
                ps = psum.tile([C, len(gs), RPG, 64], f32,
                               tag=f"ps{len(gs)}", bufs=3 if len(gs) == 2 else 2)
                for sub, g in enumerate(gs):
                    r0 = g * RPG
                    for a in range(3):
                        nc.tensor.matmul(
                            ps[:, sub, :, 0:HW],
                            wA[:, a, :],
                            A[b][:, r0 + a:r0 + a + RPG, 0:HW],
                            start=(a == 0), stop=False)
                    for a in range(3):
                        nc.tensor.matmul(
                            ps[:, sub, :, 0:HW],
                            wB[:, a, :],
                            Bp[b][:, r0 + a:r0 + a + RPG, 0:HW],
                            start=False, stop=(a == 2))
                # d = sqrt(psum + ||c||^2) (per-partition bias)
                r0 = gs[0] * RPG
                ev = nc.scalar.activation(
                    out=d[pb:pb + C, i, r0:r0 + len(gs) * RPG, :],
                    in_=ps[:, :, :, 0:HW],
                    func=AF.Sqrt, bias=cst[pb:pb + C, ci:ci + 1], scale=1.0)
                evicts.append(ev)
            # per-row batch-norm partial stats of the fresh d rows,
            # full 128-lane width (both images of the pair at once)
            for g in gs:
                col = i * NG + g
                r0 = g * RPG
                nc.vector.bn_stats(
                    out=bnt[:, col, :],
                    in_=d[:, i, r0:r0 + RPG, :].rearrange("p a b -> p (a b)"))
    return evicts


def _bn_affine(nc, pool, gstats, c2, gamma, beta, eps, name):
    """From [sum(d), sum(d^2)] global (dup both halves) -> scale s, shift t
    [128,1]. gstats[:,1] here is sum(d^2) directly (not sum(psum))."""
    P = 2 * C
    mu = pool.tile([P, 1], f32, tag=f"mu_{name}")
    ed2 = pool.tile([P, 1], f32, tag=f"ed2_{name}")
    mu2 = pool.tile([P, 1], f32, tag=f"mu2_{name}")
    var = pool.tile([P, 1], f32, tag=f"var_{name}")
    sd = pool.tile([P, 1], f32, tag=f"sd_{name}")
    inv = pool.tile([P, 1], f32, tag=f"inv_{name}")
    s = pool.tile([P, 1], f32, tag=f"s_{name}")
    st = pool.tile([P, 1], f32, tag=f"st_{name}")
    tt = pool.tile([P, 1], f32, tag=f"t_{name}")
    inv_n = 1.0 / float(N_GLOBAL)
    nc.vector.tensor_scalar_mul(out=mu[:, :], in0=gstats[:, 0:1], scalar1=inv_n)
    nc.vector.tensor_scalar_mul(out=ed2[:, :], in0=gstats[:, 1:2], scalar1=inv_n)
    nc.vector.tensor_tensor(out=mu2[:, :], in0=mu[:, :], in1=mu[:, :], op=MULT)
    nc.vector.tensor_tensor(out=var[:, :], in0=ed2[:, :], in1=mu2[:, :], op=SUB)
    nc.scalar.activation(out=sd[:, :], in_=var[:, :], func=AF.Sqrt,
                         bias=eps[:, 0:1], scale=1.0)
    nc.vector.reciprocal(out=inv[:, :], in_=sd[:, :])
    nc.vector.tensor_tensor(out=s[:, :], in0=gamma, in1=inv[:, :], op=MULT)
    nc.vector.tensor_tensor(out=st[:, :], in0=mu[:, :], in1=s[:, :], op=MULT)
    nc.vector.tensor_tensor(out=tt[:, :], in0=beta, in1=st[:, :], op=SUB)
    return s, tt


def _stats_allreduce(nc, pool, dram, bnt, name, no_collective=False):
    """bn_aggr local stats -> per-core [64,2] (sum d, sum d^2), AllGather
    across 8 cores + local rank-sum, return [128,2] duplicated global sums."""
    agg = pool.tile([2 * C, 2], f32, tag=f"agg_{name}")        # mean, var
    m2 = pool.tile([2 * C, 1], f32, tag=f"m2_{name}")
    ssq = pool.tile([2 * C, 1], f32, tag=f"ssq_{name}")
    red128 = pool.tile([2 * C, 2], f32, tag=f"red128_{name}")
    redt = pool.tile([C, 2], f32, tag=f"redt_{name}")
    red = pool.tile([C, 2], f32, tag=f"red_{name}")
    gstats = pool.tile([2 * C, 2], f32, tag=f"gstats_{name}")
    nl = float(N_LOCAL_HALF)
    nc.vector.bn_aggr(out=agg[:, :],
                      in_=bnt[:, :, :].rearrange("p a c -> p (a c)"))
    nc.vector.tensor_tensor(out=m2[:, :], in0=agg[:, 0:1], in1=agg[:, 0:1],
                            op=MULT)
    nc.vector.tensor_tensor(out=ssq[:, :], in0=agg[:, 1:2], in1=m2[:, :],
                            op=ADD)
    nc.vector.tensor_scalar_mul(out=red128[:, 0:1], in0=agg[:, 0:1], scalar1=nl)
    nc.vector.tensor_scalar_mul(out=red128[:, 1:2], in0=ssq[:, :], scalar1=nl)
    # fold images (0,1) [partitions 0:64] with (2,3) [64:128]: partition
    # bases must align within a tensor_tensor, so shift-copy first
    nc.vector.tensor_copy(out=redt[:, :], in_=red128[C:2 * C, :])
    nc.vector.tensor_tensor(out=red[:, :], in0=red128[0:C, :],
                            in1=redt[:, :], op=ADD)
    if no_collective:
        nc.vector.tensor_copy(out=gstats[0:C, :], in_=red[:, :])
        nc.vector.tensor_copy(out=gstats[C:2 * C, :], in_=gstats[0:C, :])
        return gstats
    cc_in = dram.tile([C, 2], f32, tag=f"ccin_{name}")
    # AllGather (floor ~4.6us vs AllReduce ~9.7us) + local rank-sum.
    # AG output is rank-major on the partition axis: [8*64, 2].
    cc_out = dram.tile([N_CORES * C, 2], f32, tag=f"ccout_{name}")
    gag = pool.tile([C, N_CORES, 2], f32, tag=f"gag_{name}")
    nc.sync.dma_start(out=cc_in[:, :], in_=red[:, :])
    nc.gpsimd.collective_compute(
        "AllGather", mybir.AluOpType.bypass,
        replica_groups=[list(range(N_CORES))],
        ins=[cc_in.opt()],
        outs=[cc_out.opt()],
    )
    # gag[c, r, s] <- cc_out[r*64 + c, s]
    nc.sync.dma_start(
        out=gag[:, :, :],
        in_=cc_out[:, :].rearrange("(r c) s -> c r s", r=N_CORES))
    nc.vector.tensor_reduce(out=gstats[0:C, 0:1], in_=gag[:, :, 0],
                            axis=mybir.AxisListType.X, op=ADD)
    nc.vector.tensor_reduce(out=gstats[0:C, 1:2], in_=gag[:, :, 1],
                            axis=mybir.AxisListType.X, op=ADD)
    # duplicate to the upper partition half (DVE partition-shift copy)
    nc.vector.tensor_copy(out=gstats[C:2 * C, :], in_=gstats[0:C, :])
    return gstats


def build(no_collective=False, reps=1):
    nc = bacc.Bacc("TRN2", target_bir_lowering=False, debug=False,
                   num_devices=1 if no_collective else N_CORES)
    x_ext = nc.declare_dram_parameter("x", [B_LOCAL, C, HW, HW], f32r, isOutput=False)
    wA1_ext = nc.declare_dram_parameter("wA1", [2 * C, 3, C], f32r, isOutput=False)
    wB1_ext = nc.declare_dram_parameter("wB1", [2 * C, 3, C], f16, isOutput=False)
    wA2_ext = nc.declare_dram_parameter("wA2", [2 * C, 3, C], f32r, isOutput=False)
    wB2_ext = nc.declare_dram_parameter("wB2", [2 * C, 3, C], f16, isOutput=False)
    # packed [c2a | c2b | g1 | b1 | g2 | b2], duplicated on both halves
    cst_ext = nc.declare_dram_parameter("cst", [2 * C, 6], f32, isOutput=False)
    out_ext = nc.declare_dram_parameter("out", [B_LOCAL, C, HW, HW], f32, isOutput=True)

    with tile.TileContext(nc) as tc:
        with (
            tc.tile_pool(name="big", bufs=1) as big,
            tc.tile_pool(name="sq", bufs=2) as sq,
            tc.tile_pool(name="small", bufs=1) as pool,
            tc.tile_pool(name="psum", bufs=8, space="PSUM") as psum,
            tc.tile_pool(name="dram", bufs=1, space="DRAM") as dram,
        ):
            wA1 = pool.tile([2 * C, 3, C], f32r, tag="wA1")
            wB1 = pool.tile([2 * C, 3, C], f16, tag="wB1")
            wA2 = pool.tile([2 * C, 3, C], f32r, tag="wA2")
            wB2 = pool.tile([2 * C, 3, C], f16, tag="wB2")
            cst = pool.tile([2 * C, 6], f32, tag="cst")
            g1, b1 = cst[:, 2:3], cst[:, 3:4]
            g2, b2 = cst[:, 4:5], cst[:, 5:6]
            eps = pool.tile([2 * C, 1], f32, tag="eps")
            nc.vector.memset(eps[:, :], BN_EPS)
            # constants via the gpsimd SWDGE ring (SP/ACT rings carry x)
            nc.gpsimd.dma_start(out=wA1[:, :, :], in_=wA1_ext[:, :, :])
            nc.gpsimd.dma_start(out=wB1[:, :, :], in_=wB1_ext[:, :, :])
            nc.gpsimd.dma_start(out=cst[:, :], in_=cst_ext[:, :])
            nc.gpsimd.dma_start(out=wA2[:, :, :], in_=wA2_ext[:, :, :])
            nc.gpsimd.dma_start(out=wB2[:, :, :], in_=wB2_ext[:, :, :])

            A = [big.tile([2 * C, HP, HP], f32r, tag=f"A{b}", name=f"A{b}")
                 for b in range(B_LOCAL)]
            Bp = [big.tile([2 * C, HP, HP], f16, tag=f"B{b}", name=f"B{b}")
                  for b in range(B_LOCAL)]
            d = big.tile([2 * C, 2, HW, HW], f32, tag="d")
            xres = big.tile([2 * C, 2, HW, HW], f32, tag="xres")

            for r in range(reps):
                bnt1 = pool.tile([2 * C, 2 * NG, 6], f32, tag="bnt1")
                bnt2 = pool.tile([2 * C, 2 * NG, 6], f32, tag="bnt2")

                if r == 0:
                    # zero only the plane borders once; every later write is
                    # interior-only (or copies border zeros), so the
                    # zero-padding borders persist across layers and reps.
                    # A-top needs row 0, row 57, col 0, col 57; A-bot row 0,
                    # row 57, col 56 (x01's right edge); B is fully written
                    # (cols 56,57 are never read).
                    for b in range(B_LOCAL):
                        eng = (nc.vector, nc.gpsimd)[b % 2]
                        eng.memset(A[b][:, 0:1, :].bitcast(f32), 0.0)
                        eng.memset(A[b][:, HP - 1:HP, :].bitcast(f32), 0.0)
                        eng.memset(A[b][0:C, :, 0:1].bitcast(f32), 0.0)
                        eng.memset(A[b][0:C, :, HP - 1:HP].bitcast(f32), 0.0)
                        eng.memset(A[b][C:2 * C, :, HP - 2:HP].bitcast(f32), 0.0)

                # ---- L1 planes: x, x01 direct DMA; x02/v computed ----
                # A-top rows 1..56 cols 1..56 <- x; A-bot cols 0..56 <- x01
                dma_engines = [nc.sync, nc.scalar, nc.scalar, nc.sync]
                order = (0, 2, 1, 3)
                for b in order:
                    # first pair (0,2) chunked so the first matmul groups
                    # (plane rows 0..17) are gated by a fraction of the prep
                    chunks = ((0, 18), (18, HP)) if b in (0, 2) else ((0, HP),)
                    for rr0, rr1 in chunks:
                        xr0, xr1 = max(rr0, 1) - 1, min(rr1, HW + 1) - 1
                        dma_engines[b].dma_start(
                            out=A[b][0:C, xr0 + 1:xr1 + 1, 1:HW + 1],
                            in_=x_ext[b:b + 1, :, xr0:xr1, :]
                                .transpose([1, 0, 2, 3]))
                for b in order:
                    fast = b in (0, 2)   # first pair feeds PE first
                    chunks = ((0, 18), (18, HP)) if fast else ((0, HP),)
                    s = sq.tile([2 * C, HP, HP], f16, tag="s", name=f"s{b}_l1")
                    for rr0, rr1 in chunks:
                        rr = slice(rr0, rr1)
                        # x01: on-chip shifted copy of the padded x plane
                        # (full-plane copy also renews A-bot's zero borders)
                        cp = nc.vector if fast else nc.gpsimd
                        cp.tensor_copy(out=A[b][C:2 * C, rr, 0:HP - 1],
                                       in_=A[b][0:C, rr, 1:HP])
                        # s = x^2 (fp16): borders stay 0
                        nc.scalar.activation(out=s[C:2 * C, rr, :],
                                             in_=A[b][0:C, rr, :].bitcast(f32),
                                             func=AF.Square)
                        # x02 (fp16) <- x plane shifted left 2 (border zeros
                        # ride along from the source plane)
                        cp2 = nc.vector if fast else nc.gpsimd
                        cp2.tensor_copy(out=Bp[b][0:C, rr, 0:HW],
                                        in_=A[b][0:C, rr, 2:HP].bitcast(f32))
                        # v = s + s01 + s02 (3-tap row sum, fp16 2x DVE)
                        nc.vector.tensor_tensor(
                            out=Bp[b][C:2 * C, rr, 0:HW],
                            in0=s[C:2 * C, rr, 0:HW],
                            in1=s[C:2 * C, rr, 1:HW + 1], op=ADD)
                        nc.vector.tensor_tensor(
                            out=Bp[b][C:2 * C, rr, 0:HW],
                            in0=Bp[b][C:2 * C, rr, 0:HW],
                            in1=s[C:2 * C, rr, 2:HP], op=ADD)

                # ---- layer 1 ----
                ev1 = _build_layer(nc, psum, A, Bp, wA1, wB1, cst, 0, d, bnt1)

                # residual copy of x, 128-wide layout: needed only at the very
                # end; order it after the L1 evictions start (DMA engines idle
                # mid-layer; keeps it off the startup critical path)
                for b in range(B_LOCAL):
                    pb, i = 64 * (b // 2), b % 2
                    xr = nc.gpsimd.dma_start(
                        out=xres[pb:pb + C, i:i + 1, :, :],
                        in_=x_ext[b:b + 1, :, :, :].transpose([1, 0, 2, 3])
                            .bitcast(f32))
                    tile.add_dep_helper(xr.ins, ev1[(4 * b) % 16].ins,
                                        reason="defer xres DMA past L1 start")
                gstats1 = _stats_allreduce(nc, pool, dram, bnt1, "l1",
                                           no_collective)
                s1, t1 = _bn_affine(nc, pool, gstats1, cst[:, 0:1], g1, b1,
                                    eps, "l1")

                # ---- L2 planes: y = relu(s1*d + t1) into the same buffers --
                for b in order:
                    pb, i = 64 * (b // 2), b % 2
                    # y (f32r) into A-top interior; borders still zero
                    nc.scalar.activation(
                        out=A[b][0:C, 1:HW + 1, 1:HW + 1],
                        in_=d[pb:pb + C, i, :, :],
                        func=AF.Relu, bias=t1[pb:pb + C, 0:1],
                        scale=s1[pb:pb + C, 0:1])
                    # y01 (f32r copy, DVE dual-port) incl border zeros
                    nc.vector.tensor_copy(
                        out=A[b][C:2 * C, :, 0:HP - 1],
                        in_=A[b][0:C, :, 1:HP])
                    # y02 (fp16 convert); gpsimd for the trailing pair
                    cp2 = nc.vector if b in (0, 2) else nc.gpsimd
                    cp2.tensor_copy(out=Bp[b][0:C, :, 0:HW],
                                    in_=A[b][0:C, :, 2:HP].bitcast(f32))
                    s = sq.tile([2 * C, HP, HP], f16, tag="s", name=f"s{b}_l2")
                    nc.scalar.activation(out=s[C:2 * C, :, :],
                                         in_=A[b][0:C, :, :].bitcast(f32),
                                         func=AF.Square)
                    nc.vector.tensor_tensor(
                        out=Bp[b][C:2 * C, :, 0:HW], in0=s[C:2 * C, :, 0:HW],
                        in1=s[C:2 * C, :, 1:HW + 1], op=ADD)
                    nc.vector.tensor_tensor(
                        out=Bp[b][C:2 * C, :, 0:HW],
                        in0=Bp[b][C:2 * C, :, 0:HW],
                        in1=s[C:2 * C, :, 2:HP], op=ADD)

                # ---- layer 2 ----
                _build_layer(nc, psum, A, Bp, wA2, wB2, cst, 1, d, bnt2)
                gstats2 = _stats_allreduce(nc, pool, dram, bnt2, "l2",
                                           no_collective)
                s2, t2 = _bn_affine(nc, pool, gstats2, cst[:, 1:2], g2, b2,
                                    eps, "l2")

                # ---- final: out = relu(s2*d + t2 + x), 128-wide; DMA out ---
                for i in range(2):
                    for su in range(2):
                        rows = su * (HW // 2)
                        rs = slice(rows, rows + HW // 2)
                        nc.vector.scalar_tensor_tensor(
                            out=d[:, i, rs, :], in0=d[:, i, rs, :],
                            scalar=s2[:, 0:1], in1=xres[:, i, rs, :],
                            op0=MULT, op1=ADD)
                        nc.scalar.activation(
                            out=d[:, i, rs, :], in_=d[:, i, rs, :],
                            func=AF.Relu, bias=t2[:, 0:1], scale=1.0)
                        for h in range(2):
                            b = 2 * h + i
                            nc.vector.dma_start(
                                out=out_ext[b:b + 1, :, rs, :].transpose(
                                    [1, 0, 2, 3]),
                                in_=d[64 * h:64 * h + C, i:i + 1, rs, :])
    nc.compile()
    return nc


_NC_CACHE = None


def _get_nc():
    global _NC_CACHE
    if _NC_CACHE is None:
        _NC_CACHE = build()
    return _NC_CACHE


def _make_in_maps(x, centers1, gamma1, beta1, centers2, gamma2, beta2):
    def prep_w(centers):
        # centers: [o, dd] with dd = c*9 + a*3 + kw
        ct = -2.0 * np.ascontiguousarray(
            centers.reshape(C, C, 3, 3).transpose(1, 2, 3, 0))  # [c, a, kw, o]
        wA = np.empty((2 * C, 3, C), np.float32)
        wA[:C] = ct[:, :, 0, :]       # taps (a, 0)
        wA[C:] = ct[:, :, 1, :]       # taps (a, 1)
        wB = np.empty((2 * C, 3, C), np.float16)
        wB[:C] = ct[:, :, 2, :]       # taps (a, 2)
        wB[C:] = 1.0                  # ones: sum v -> ||p||^2
        return wA, wB

    c1 = np.asarray(centers1, np.float32)
    c2 = np.asarray(centers2, np.float32)
    wA1, wB1 = prep_w(c1)
    wA2, wB2 = prep_w(c2)
    cst = np.stack([
        (c1 ** 2).sum(1), (c2 ** 2).sum(1),
        np.asarray(gamma1, np.float32), np.asarray(beta1, np.float32),
        np.asarray(gamma2, np.float32), np.asarray(beta2, np.float32),
    ], axis=1).astype(np.float32)
    cst = np.ascontiguousarray(np.tile(cst, (2, 1)))   # duplicate both halves
    common = {
        "wA1": wA1, "wB1": wB1, "wA2": wA2, "wB2": wB2,
        "cst": cst,
    }
    x = np.asarray(x, np.float32)
    in_maps = []
    for c in range(N_CORES):
        m = dict(common)
        m["x"] = np.ascontiguousarray(x[c * B_LOCAL:(c + 1) * B_LOCAL])
        in_maps.append(m)
    return in_maps


def _run(inputs, trace=False, **kw):
    nc = _get_nc()
    in_maps = _make_in_maps(**inputs)
    res = run_bass_kernel_spmd(nc, in_maps, core_ids=list(range(N_CORES)),
                               trace=trace, **kw)
    out = np.concatenate([res.results[c]["out"] for c in range(N_CORES)], axis=0)
    return out.astype(np.float32), res


def kernel(**inputs):
    out, _ = _run(inputs)
    return out


# revision 28
# speedup vs baseline: 1.1875x; 1.1875x over previous
"""Trainium2 Bass kernel for nn_BasicBlock (distance-transform conv BasicBlock).

Computes: relu(bn2(dt_conv2(relu(bn1(dt_conv1(x))))) + x)
where dt_conv is a 3x3 "distance transform conv":
    d[b,o,h,w] = sqrt(||p - c_o||^2),  p = 3x3 zero-padded patch (dim 576)

Strategy (8 NeuronCores, data-parallel over batch 32 -> 4 images/core):

6 matmuls per pixel-group instead of 9 (1.5x less PE work). With
||p||^2 - 2 p.c + ||c||^2, the -2p.c part needs all 9 taps x 64 ch = 576
contraction lanes, but the ||p||^2 part is predigested: v = row3(x^2)
(3-tap horizontal sum of the per-channel square, on DVE) compresses the
576 norm lanes into 192. Two SBUF planes per image:
  A = [x | x01]  (f32r, x01 = x shifted left 1 col; both direct HBM DMAs)
  B = [x02 | v]  (fp16)
Six matmuls per group, all at window column 0, rows a=0..2:
  A windows: top-> tap(a,0) w/ centers, bottom-> tap(a,1) w/ centers
  B windows: top-> tap(a,2) w/ centers, bottom-> v w/ ones (sums to ||p||^2)
K=128 fully packed: 576 + 192 = 768 = 6 x 128.

Images b and b+2 share one PSUM bank (PE tile_position col 0 / col 64), so
evictions (ACT sqrt+bias) run at full 128-lane width, N=448 (8 rows x 56).
BN batch stats via bn_stats/bn_aggr on the evicted d chunks (DVE), fold,
[64,2] AllGather across 8 cores + local rank-sum (sync-BN), x2 layers.
Layer 2 reuses A/B buffers: y (ACT glue, f32), y01 (DVE copy), y02
(GPSIMD convert-copy), v' = row3(y^2) (ACT square + DVE adds). Zero
borders are established once (rep 0) and preserved by interior-only
writes. Final: out = relu(s2*d + t2 + x): GPSIMD stt + ACT relu + DMA.

kernel(**inputs) takes FULL unsharded inputs, returns FULL output.
Self-contained: shapes/sharding hardcoded; no file reads.
"""
import numpy as np

from concourse import bacc, mybir, tile
from concourse.bass_utils import run_bass_kernel_spmd

f32 = mybir.dt.float32
f32r = mybir.dt.float32r
f16 = mybir.dt.float16
ADD = mybir.AluOpType.add
MULT = mybir.AluOpType.mult
SUB = mybir.AluOpType.subtract
AF = mybir.ActivationFunctionType

N_CORES = 8
B_LOCAL = 4            # images per core (32 / 8)
C = 64                 # channels (in == out)
HW = 56                # spatial
HP = HW + 2            # padded plane side
RPG = 8                # rows per matmul group (N = 8*56 = 448)
NG = 7                 # groups per image-pair (7*8 = 56 rows)
N_GLOBAL = 32 * HW * HW             # BN normalization count (global batch)
N_LOCAL_HALF = 2 * HW * HW          # values per partition in bn_aggr (2 imgs)
BN_EPS = 1e-5


def _build_layer(nc, psum, A, Bp, wA, wB, cst, ci, d, bnt):
    """One dt_conv layer: 14 full-bank groups x 2x6 window matmuls (N=448).
    A[b] f32r [128, 58, 58] = [x | x01]; Bp[b] f16 [128, 58, 58] = [x02 | v].
    Images b=i and b=2+i share a PSUM bank (partitions 0:64 / 64:128).
    d is [128, 2, HW, HW]; bnt [128, 14, 6] per-chunk bn stats.
    Returns eviction instructions (scheduling anchors)."""
    evicts = []
    # groups fused in pairs per PSUM tile (2 banks) so each ACT eviction
    # covers 16 rows; 7 groups -> (0,1) (2,3) (4,5) (6)
    fused = [(0, 1), (2, 3), (4, 5), (6,)]
    for i in range(2):              # image-pair index (b = 2*h + i)
        for gs in fused:
            for h in range(2):
                b = 2 * h + i
                pb = 64 * h
                # matmul output must start at PSUM partition 0 (ISA rule),
                # so each image gets 64-wide banks; the eviction shifts
                # partitions (ACT out base pb) into the 128-wide d.
                ps = psum.tile([C, len(gs), RPG, 64], f32,
                               tag=f"ps{len(gs)}", bufs=3 if len(gs) == 2 else 2)
                for sub, g in enumerate(gs):
                    r0 = g * RPG
                    for a in range(3):
                        nc.tensor.matmul(
                            ps[:, sub, :, 0:HW],
                            wA[:, a, :],
                            A[b][:, r0 + a:r0 + a + RPG, 0:HW],
                            start=(a == 0), stop=False)
                    for a in range(3):
                        nc.tensor.matmul(
                            ps[:, sub, :, 0:HW],
                            wB[:, a, :],
                            Bp[b][:, r0 + a:r0 + a + RPG, 0:HW],
                            start=False, stop=(a == 2))
                # d = sqrt(psum + ||c||^2) (per-partition bias)
                r0 = gs[0] * RPG
                ev = nc.scalar.activation(
                    out=d[pb:pb + C, i, r0:r0 + len(gs) * RPG, :],
                    in_=ps[:, :, :, 0:HW],
                    func=AF.Sqrt, bias=cst[pb:pb + C, ci:ci + 1], scale=1.0)
                evicts.append(ev)
            # per-row batch-norm partial stats of the fresh d rows,
            # full 128-lane width (both images of the pair at once)
            for g in gs:
                col = i * NG + g
                r0 = g * RPG
                nc.vector.bn_stats(
                    out=bnt[:, col, :],
                    in_=d[:, i, r0:r0 + RPG, :].rearrange("p a b -> p (a b)"))
    return evicts


def _bn_affine(nc, pool, gstats, c2, gamma, beta, eps, name):
    """From [sum(d), sum(d^2)] global (dup both halves) -> scale s, shift t
    [128,1]. gstats[:,1] here is sum(d^2) directly (not sum(psum))."""
    P = 2 * C
    mu = pool.tile([P, 1], f32, tag=f"mu_{name}")
    ed2 = pool.tile([P, 1], f32, tag=f"ed2_{name}")
    mu2 = pool.tile([P, 1], f32, tag=f"mu2_{name}")
    var = pool.tile([P, 1], f32, tag=f"var_{name}")
    sd = pool.tile([P, 1], f32, tag=f"sd_{name}")
    inv = pool.tile([P, 1], f32, tag=f"inv_{name}")
    s = pool.tile([P, 1], f32, tag=f"s_{name}")
    st = pool.tile([P, 1], f32, tag=f"st_{name}")
    tt = pool.tile([P, 1], f32, tag=f"t_{name}")
    inv_n = 1.0 / float(N_GLOBAL)
    nc.vector.tensor_scalar_mul(out=mu[:, :], in0=gstats[:, 0:1], scalar1=inv_n)
    nc.vector.tensor_scalar_mul(out=ed2[:, :], in0=gstats[:, 1:2], scalar1=inv_n)
    nc.vector.tensor_tensor(out=mu2[:, :], in0=mu[:, :], in1=mu[:, :], op=MULT)
    nc.vector.tensor_tensor(out=var[:, :], in0=ed2[:, :], in1=mu2[:, :], op=SUB)
    nc.scalar.activation(out=sd[:, :], in_=var[:, :], func=AF.Sqrt,
                         bias=eps[:, 0:1], scale=1.0)
    nc.vector.reciprocal(out=inv[:, :], in_=sd[:, :])
    nc.vector.tensor_tensor(out=s[:, :], in0=gamma, in1=inv[:, :], op=MULT)
    nc.vector.tensor_tensor(out=st[:, :], in0=mu[:, :], in1=s[:, :], op=MULT)
    nc.vector.tensor_tensor(out=tt[:, :], in0=beta, in1=st[:, :], op=SUB)
    return s, tt


def _stats_allreduce(nc, pool, dram, bnt, name, no_collective=False):
    """bn_aggr local stats -> per-core [64,2] (sum d, sum d^2), AllGather
    across 8 cores + local rank-sum, return [128,2] duplicated global sums."""
    agg = pool.tile([2 * C, 2], f32, tag=f"agg_{name}")        # mean, var
    m2 = pool.tile([2 * C, 1], f32, tag=f"m2_{name}")
    ssq = pool.tile([2 * C, 1], f32, tag=f"ssq_{name}")
    red128 = pool.tile([2 * C, 2], f32, tag=f"red128_{name}")
    redt = pool.tile([C, 2], f32, tag=f"redt_{name}")
    red = pool.tile([C, 2], f32, tag=f"red_{name}")
    gstats = pool.tile([2 * C, 2], f32, tag=f"gstats_{name}")
    nl = float(N_LOCAL_HALF)
    nc.vector.bn_aggr(out=agg[:, :],
                      in_=bnt[:, :, :].rearrange("p a c -> p (a c)"))
    nc.vector.tensor_tensor(out=m2[:, :], in0=agg[:, 0:1], in1=agg[:, 0:1],
                            op=MULT)
    nc.vector.tensor_tensor(out=ssq[:, :], in0=agg[:, 1:2], in1=m2[:, :],
                            op=ADD)
    nc.vector.tensor_scalar_mul(out=red128[:, 0:1], in0=agg[:, 0:1], scalar1=nl)
    nc.vector.tensor_scalar_mul(out=red128[:, 1:2], in0=ssq[:, :], scalar1=nl)
    # fold images (0,1) [partitions 0:64] with (2,3) [64:128]: partition
    # bases must align within a tensor_tensor, so shift-copy first
    nc.vector.tensor_copy(out=redt[:, :], in_=red128[C:2 * C, :])
    nc.vector.tensor_tensor(out=red[:, :], in0=red128[0:C, :],
                            in1=redt[:, :], op=ADD)
    if no_collective:
        nc.vector.tensor_copy(out=gstats[0:C, :], in_=red[:, :])
        nc.vector.tensor_copy(out=gstats[C:2 * C, :], in_=gstats[0:C, :])
        return gstats
    cc_in = dram.tile([C, 2], f32, tag=f"ccin_{name}")
    # AllGather (floor ~4.6us vs AllReduce ~9.7us) + local rank-sum.
    # AG output is rank-major on the partition axis: [8*64, 2].
    cc_out = dram.tile([N_CORES * C, 2], f32, tag=f"ccout_{name}")
    gag = pool.tile([C, N_CORES, 2], f32, tag=f"gag_{name}")
    nc.sync.dma_start(out=cc_in[:, :], in_=red[:, :])
    nc.gpsimd.collective_compute(
        "AllGather", mybir.AluOpType.bypass,
        replica_groups=[list(range(N_CORES))],
        ins=[cc_in.opt()],
        outs=[cc_out.opt()],
    )
    # gag[c, r, s] <- cc_out[r*64 + c, s]
    nc.sync.dma_start(
        out=gag[:, :, :],
        in_=cc_out[:, :].rearrange("(r c) s -> c r s", r=N_CORES))
    nc.vector.tensor_reduce(out=gstats[0:C, 0:1], in_=gag[:, :, 0],
                            axis=mybir.AxisListType.X, op=ADD)
    nc.vector.tensor_reduce(out=gstats[0:C, 1:2], in_=gag[:, :, 1],
                            axis=mybir.AxisListType.X, op=ADD)
    # duplicate to the upper partition half (DVE partition-shift copy)
    nc.vector.tensor_copy(out=gstats[C:2 * C, :], in_=gstats[0:C, :])
    return gstats


def build(no_collective=False, reps=1):
    nc = bacc.Bacc("TRN2", target_bir_lowering=False, debug=False,
                   num_devices=1 if no_collective else N_CORES)
    x_ext = nc.declare_dram_parameter("x", [B_LOCAL, C, HW, HW], f32r, isOutput=False)
    wA1_ext = nc.declare_dram_parameter("wA1", [2 * C, 3, C], f32r, isOutput=False)
    wB1_ext = nc.declare_dram_parameter("wB1", [2 * C, 3, C], f16, isOutput=False)
    wA2_ext = nc.declare_dram_parameter("wA2", [2 * C, 3, C], f32r, isOutput=False)
    wB2_ext = nc.declare_dram_parameter("wB2", [2 * C, 3, C], f16, isOutput=False)
    # packed [c2a | c2b | g1 | b1 | g2 | b2], duplicated on both halves
    cst_ext = nc.declare_dram_parameter("cst", [2 * C, 6], f32, isOutput=False)
    out_ext = nc.declare_dram_parameter("out", [B_LOCAL, C, HW, HW], f32, isOutput=True)

    with tile.TileContext(nc) as tc:
        with (
            tc.tile_pool(name="big", bufs=1) as big,
            tc.tile_pool(name="sq", bufs=2) as sq,
            tc.tile_pool(name="small", bufs=1) as pool,
            tc.tile_pool(name="psum", bufs=8, space="PSUM") as psum,
            tc.tile_pool(name="dram", bufs=1, space="DRAM") as dram,
        ):
            wA1 = pool.tile([2 * C, 3, C], f32r, tag="wA1")
            wB1 = pool.tile([2 * C, 3, C], f16, tag="wB1")
            wA2 = pool.tile([2 * C, 3, C], f32r, tag="wA2")
            wB2 = pool.tile([2 * C, 3, C], f16, tag="wB2")
            cst = pool.tile([2 * C, 6], f32, tag="cst")
            g1, b1 = cst[:, 2:3], cst[:, 3:4]
            g2, b2 = cst[:, 4:5], cst[:, 5:6]
            eps = pool.tile([2 * C, 1], f32, tag="eps")
            nc.vector.memset(eps[:, :], BN_EPS)
            # constants via the gpsimd SWDGE ring (SP/ACT rings carry x)
            nc.gpsimd.dma_start(out=wA1[:, :, :], in_=wA1_ext[:, :, :])
            nc.gpsimd.dma_start(out=wB1[:, :, :], in_=wB1_ext[:, :, :])
            nc.gpsimd.dma_start(out=cst[:, :], in_=cst_ext[:, :])
            nc.gpsimd.dma_start(out=wA2[:, :, :], in_=wA2_ext[:, :, :])
            nc.gpsimd.dma_start(out=wB2[:, :, :], in_=wB2_ext[:, :, :])

            A = [big.tile([2 * C, HP, HP], f32r, tag=f"A{b}", name=f"A{b}")
                 for b in range(B_LOCAL)]
            Bp = [big.tile([2 * C, HP, HP], f16, tag=f"B{b}", name=f"B{b}")
                  for b in range(B_LOCAL)]
            d = big.tile([2 * C, 2, HW, HW], f32, tag="d")
            xres = big.tile([2 * C, 2, HW, HW], f32, tag="xres")

            for r in range(reps):
                bnt1 = pool.tile([2 * C, 2 * NG, 6], f32, tag="bnt1")
                bnt2 = pool.tile([2 * C, 2 * NG, 6], f32, tag="bnt2")

                if r == 0:
                    # zero only the plane borders once; every later write is
                    # interior-only (or copies border zeros), so the
                    # zero-padding borders persist across layers and reps.
                    # A-top needs row 0, row 57, col 0, col 57; A-bot row 0,
                    # row 57, col 56 (x01's right edge); B is fully written
                    # (cols 56,57 are never read).
                    for b in range(B_LOCAL):
                        eng = (nc.vector, nc.gpsimd)[b % 2]
                        eng.memset(A[b][:, 0:1, :].bitcast(f32), 0.0)
                        eng.memset(A[b][:, HP - 1:HP, :].bitcast(f32), 0.0)
                        eng.memset(A[b][0:C, :, 0:1].bitcast(f32), 0.0)
                        eng.memset(A[b][0:C, :, HP - 1:HP].bitcast(f32), 0.0)
                        eng.memset(A[b][C:2 * C, :, HP - 2:HP].bitcast(f32), 0.0)

                # ---- L1 planes: x, x01 direct DMA; x02/v computed ----
                # A-top rows 1..56 cols 1..56 <- x; A-bot cols 0..56 <- x01
                dma_engines = [nc.sync, nc.sync, nc.sync, nc.sync]
                order = (0, 2, 1, 3)
                for b in order:
                    # first pair (0,2) chunked so the first matmul groups
                    # (plane rows 0..17) are gated by a fraction of the prep
                    chunks = ((0, 18), (18, HP)) if b in (0, 2) else ((0, HP),)
                    for rr0, rr1 in chunks:
                        xr0, xr1 = max(rr0, 1) - 1, min(rr1, HW + 1) - 1
                        dma_engines[b].dma_start(
                            out=A[b][0:C, xr0 + 1:xr1 + 1, 1:HW + 1],
                            in_=x_ext[b:b + 1, :, xr0:xr1, :]
                                .transpose([1, 0, 2, 3]))
                for b in order:
                    fast = b in (0, 2)   # first pair feeds PE first
                    chunks = ((0, 18), (18, HP)) if fast else ((0, HP),)
                    s = sq.tile([2 * C, HP, HP], f16, tag="s", name=f"s{b}_l1")
                    for rr0, rr1 in chunks:
                        rr = slice(rr0, rr1)
                        # x01: on-chip shifted copy of the padded x plane
                        # (full-plane copy also renews A-bot's zero borders)
                        cp = nc.vector
                        cp.tensor_copy(out=A[b][C:2 * C, rr, 0:HP - 1],
                                       in_=A[b][0:C, rr, 1:HP])
                        # s = x^2 (fp16): borders stay 0
                        nc.scalar.activation(out=s[C:2 * C, rr, :],
                                             in_=A[b][0:C, rr, :].bitcast(f32),
                                             func=AF.Square)
                        # x02 (fp16) <- x plane shifted left 2 (border zeros
                        # ride along from the source plane)
                        cp2 = nc.vector
                        cp2.tensor_copy(out=Bp[b][0:C, rr, 0:HW],
                                        in_=A[b][0:C, rr, 2:HP].bitcast(f32))
                        # v = s + s01 + s02 (3-tap row sum, fp16 2x DVE)
                        nc.vector.tensor_tensor(
                            out=Bp[b][C:2 * C, rr, 0:HW],
                            in0=s[C:2 * C, rr, 0:HW],
                            in1=s[C:2 * C, rr, 1:HW + 1], op=ADD)
                        nc.vector.tensor_tensor(
                            out=Bp[b][C:2 * C, rr, 0:HW],
                            in0=Bp[b][C:2 * C, rr, 0:HW],
                            in1=s[C:2 * C, rr, 2:HP], op=ADD)

                # ---- layer 1 ----
                ev1 = _build_layer(nc, psum, A, Bp, wA1, wB1, cst, 0, d, bnt1)

                # residual copy of x, 128-wide layout: needed only at the very
                # end; order it after the L1 evictions start (DMA engines idle
                # mid-layer; keeps it off the startup critical path)
                for b in range(B_LOCAL):
                    pb, i = 64 * (b // 2), b % 2
                    xr = nc.gpsimd.dma_start(
                        out=xres[pb:pb + C, i:i + 1, :, :],
                        in_=x_ext[b:b + 1, :, :, :].transpose([1, 0, 2, 3])
                            .bitcast(f32))
                    tile.add_dep_helper(xr.ins, ev1[(4 * b) % 16].ins,
                                        reason="defer xres DMA past L1 start")
                gstats1 = _stats_allreduce(nc, pool, dram, bnt1, "l1",
                                           no_collective)
                s1, t1 = _bn_affine(nc, pool, gstats1, cst[:, 0:1], g1, b1,
                                    eps, "l1")

                # ---- L2 planes: y = relu(s1*d + t1) into the same buffers --
                for b in order:
                    pb, i = 64 * (b // 2), b % 2
                    # y (f32r) into A-top interior; borders still zero
                    nc.scalar.activation(
                        out=A[b][0:C, 1:HW + 1, 1:HW + 1],
                        in_=d[pb:pb + C, i, :, :],
                        func=AF.Relu, bias=t1[pb:pb + C, 0:1],
                        scale=s1[pb:pb + C, 0:1])
                    # y01 (f32r copy, DVE dual-port) incl border zeros
                    nc.vector.tensor_copy(
                        out=A[b][C:2 * C, :, 0:HP - 1],
                        in_=A[b][0:C, :, 1:HP])
                    # y02 (fp16 convert); gpsimd for the trailing pair
                    cp2 = nc.vector
                    cp2.tensor_copy(out=Bp[b][0:C, :, 0:HW],
                                    in_=A[b][0:C, :, 2:HP].bitcast(f32))
                    s = sq.tile([2 * C, HP, HP], f16, tag="s", name=f"s{b}_l2")
                    nc.scalar.activation(out=s[C:2 * C, :, :],
                                         in_=A[b][0:C, :, :].bitcast(f32),
                                         func=AF.Square)
                    nc.vector.tensor_tensor(
                        out=Bp[b][C:2 * C, :, 0:HW], in0=s[C:2 * C, :, 0:HW],
                        in1=s[C:2 * C, :, 1:HW + 1], op=ADD)
                    nc.vector.tensor_tensor(
                        out=Bp[b][C:2 * C, :, 0:HW],
                        in0=Bp[b][C:2 * C, :, 0:HW],
                        in1=s[C:2 * C, :, 2:HP], op=ADD)

                # ---- layer 2 ----
                _build_layer(nc, psum, A, Bp, wA2, wB2, cst, 1, d, bnt2)
                gstats2 = _stats_allreduce(nc, pool, dram, bnt2, "l2",
                                           no_collective)
                s2, t2 = _bn_affine(nc, pool, gstats2, cst[:, 1:2], g2, b2,
                                    eps, "l2")

                # ---- final: out = relu(s2*d + t2 + x), 128-wide; DMA out ---
                for i in range(2):
                    for su in range(2):
                        rows = su * (HW // 2)
                        rs = slice(rows, rows + HW // 2)
                        nc.vector.scalar_tensor_tensor(
                            out=d[:, i, rs, :], in0=d[:, i, rs, :],
                            scalar=s2[:, 0:1], in1=xres[:, i, rs, :],
                            op0=MULT, op1=ADD)
                        nc.scalar.activation(
                            out=d[:, i, rs, :], in_=d[:, i, rs, :],
                            func=AF.Relu, bias=t2[:, 0:1], scale=1.0)
                        for h in range(2):
                            b = 2 * h + i
                            nc.scalar.dma_start(
                                out=out_ext[b:b + 1, :, rs, :].transpose(
                                    [1, 0, 2, 3]),
                                in_=d[64 * h:64 * h + C, i:i + 1, rs, :])
    nc.compile()
    return nc


_NC_CACHE = None


def _get_nc():
    global _NC_CACHE
    if _NC_CACHE is None:
        _NC_CACHE = build()
    return _NC_CACHE


def _make_in_maps(x, centers1, gamma1, beta1, centers2, gamma2, beta2):
    def prep_w(centers):
        # centers: [o, dd] with dd = c*9 + a*3 + kw
        ct = -2.0 * np.ascontiguousarray(
            centers.reshape(C, C, 3, 3).transpose(1, 2, 3, 0))  # [c, a, kw, o]
        wA = np.empty((2 * C, 3, C), np.float32)
        wA[:C] = ct[:, :, 0, :]       # taps (a, 0)
        wA[C:] = ct[:, :, 1, :]       # taps (a, 1)
        wB = np.empty((2 * C, 3, C), np.float16)
        wB[:C] = ct[:, :, 2, :]       # taps (a, 2)
        wB[C:] = 1.0                  # ones: sum v -> ||p||^2
        return wA, wB

    c1 = np.asarray(centers1, np.float32)
    c2 = np.asarray(centers2, np.float32)
    wA1, wB1 = prep_w(c1)
    wA2, wB2 = prep_w(c2)
    cst = np.stack([
        (c1 ** 2).sum(1), (c2 ** 2).sum(1),
        np.asarray(gamma1, np.float32), np.asarray(beta1, np.float32),
        np.asarray(gamma2, np.float32), np.asarray(beta2, np.float32),
    ], axis=1).astype(np.float32)
    cst = np.ascontiguousarray(np.tile(cst, (2, 1)))   # duplicate both halves
    common = {
        "wA1": wA1, "wB1": wB1, "wA2": wA2, "wB2": wB2,
        "cst": cst,
    }
    x = np.asarray(x, np.float32)
    in_maps = []
    for c in range(N_CORES):
        m = dict(common)
        m["x"] = np.ascontiguousarray(x[c * B_LOCAL:(c + 1) * B_LOCAL])
        in_maps.append(m)
    return in_maps


def _run(inputs, trace=False, **kw):
    nc = _get_nc()
    in_maps = _make_in_maps(**inputs)
    res = run_bass_kernel_spmd(nc, in_maps, core_ids=list(range(N_CORES)),
                               trace=trace, **kw)
    out = np.concatenate([res.results[c]["out"] for c in range(N_CORES)], axis=0)
    return out.astype(np.float32), res


def kernel(**inputs):
    out, _ = _run(inputs)
    return out
